# revision 13
# baseline (speedup 1.0000x reference)
"""Causal self-attention (B=4, T=2048, C=1024, H=16) on 8 TRN2 NeuronCores.

Sharding: core = 2*b + parity. Each core handles batch b's queries at
tokens parity::2 (1024 queries). K/V are computed for the full 2048-token
context (redundantly per batch pair) so no collectives are needed, and the
even/odd interleave makes the causal block structure identical on every
core: local query sub-block m (128 queries) attends exactly key blocks
0..2m+1, with a single shared [128(key),128(query)] diagonal mask per
parity applied to the last two key blocks.

v2 restructure (one continuous PE river, all-bf16 inputs):
  P0: K+V projections for tokens 0..1023 (DMA streamed in first-use order)
  P1: Q projection (all 1024 queries)
  S3: 16 j0 attention steps, K+V projections for tokens 1024..2047
      interleaved as PE filler (j0 steps alone are ScalarE-bound)
  S4: 16 j1 attention steps, j0 output projection as PE filler
  S5: j1 output projection tail (double-buffered PSUM)

Per step s: scores(head s) interleave with AV(head s-1) on the PE;
exp on ScalarE chases the scores; causal masks on GpSimd right after the
exp chunk they need (AV of the next step never waits); softmax
normalization is PE/ScalarE-free: reciprocal_approx_fast (DVE) on the
denominator row + partition_broadcast (GpSimd) + one DVE multiply.
"""

import math
from contextlib import ExitStack

import numpy as np

B, T, C, H = 4, 2048, 1024, 16
D = C // H  # 64
P = 128
N_CORES = 8
NKB = T // P  # 16 key blocks of 128
TQ = T // 2  # 1024 queries per core
SCALE = 1.0 / math.sqrt(D)
CB = C // P  # 8 channel blocks
TC = 512  # token chunk for projections

_CACHE = {}


def _build_nc():
    import concourse.tile as tile
    from concourse import bacc, mybir
    from concourse.bass_interp import get_hw_module
    from concourse import hw_specs

    if not getattr(bacc, "_attn_act_tbl_patch", False):
        _orig_tables = hw_specs.get_activation_tables

        def _tables_exp_with_ln(arch):
            t = _orig_tables(arch)
            for name, fns in t.items():
                if name != "natural_log_exp_and_others":
                    fns.discard(mybir.ActivationFunctionType.Exp)
            return t

        bacc.get_activation_tables = _tables_exp_with_ln
        bacc._attn_act_tbl_patch = True

    f32 = mybir.dt.float32
    bf16 = mybir.dt.bfloat16

    nc = bacc.Bacc("TRN2", target_bir_lowering=False, debug=False,
                   num_devices=N_CORES)

    xctxT = nc.dram_tensor("xctxT", [C, T], bf16, kind="ExternalInput").ap()
    xqT = nc.dram_tensor("xqT", [C, TQ], bf16, kind="ExternalInput").ap()
    Wq = nc.dram_tensor("Wq", [C, C], bf16, kind="ExternalInput").ap()
    Wk = nc.dram_tensor("Wk", [C, C], bf16, kind="ExternalInput").ap()
    Wv = nc.dram_tensor("Wv", [C, C], bf16, kind="ExternalInput").ap()
    Wp = nc.dram_tensor("Wp", [C, C], bf16, kind="ExternalInput").ap()
    bq = nc.dram_tensor("bq", [P, CB], f32, kind="ExternalInput").ap()
    bk = nc.dram_tensor("bk", [P, CB], f32, kind="ExternalInput").ap()
    bp = nc.dram_tensor("bp", [P, CB], f32, kind="ExternalInput").ap()
    vbias = nc.dram_tensor("vbias", [P, H, D], bf16, kind="ExternalInput").ap()
    maskT = nc.dram_tensor("maskT", [P, 2, P], bf16, kind="ExternalInput").ap()
    outT = nc.dram_tensor("outT", [C, TQ], f32, kind="ExternalOutput").ap()

    with tile.TileContext(nc) as tc, ExitStack() as top:
        persist = top.enter_context(tc.tile_pool(name="persist", bufs=1))
        small = top.enter_context(tc.tile_pool(name="small", bufs=1))
        xin = top.enter_context(tc.tile_pool(name="xin", bufs=3))
        wpp = top.enter_context(tc.tile_pool(name="wpp", bufs=3))
        ppool = top.enter_context(tc.tile_pool(name="ppool", bufs=2))
        ypool = top.enter_context(tc.tile_pool(name="ypool", bufs=2))
        opool = top.enter_context(tc.tile_pool(name="opool", bufs=2))
        nrm = top.enter_context(tc.tile_pool(name="nrm", bufs=2))
        bcp = top.enter_context(tc.tile_pool(name="bcp", bufs=1))
        pmm = top.enter_context(tc.tile_pool(name="pmm", bufs=2, space="PSUM"))
        ps_s = top.enter_context(
            tc.tile_pool(name="ps_s", bufs=2, space="PSUM"))
        ps_y = top.enter_context(
            tc.tile_pool(name="ps_y", bufs=2, space="PSUM"))

        # persistent SBUF tensors
        kT_sb = persist.tile([P, CB, T], bf16, tag="kT")
        v_sb = persist.tile([P, H, NKB, D + 1], bf16, tag="v")
        qT_sb = persist.tile([P, CB, TQ], bf16, tag="qT")
        wk_sb = persist.tile([P, CB, C], bf16, tag="wk")
        wv_sb = persist.tile([P, CB, C], bf16, tag="wv")

        bq_sb = small.tile([P, CB], f32, tag="bq")
        bk_sb = small.tile([P, CB], f32, tag="bk")
        bp_sb = small.tile([P, CB], f32, tag="bp")
        vb_sb = small.tile([P, H, D], bf16, tag="vb")
        mask_sb = small.tile([P, 2, P], bf16, tag="mask")

        # ---- input DMA, ordered by first use ----
        # sync queue: weights (Wk first; first matmul needs only Wk + x0)
        nc.sync.dma_start(wk_sb[:], Wk.rearrange("(o p) c -> p o c", p=P))
        nc.sync.dma_start(bk_sb[:], bk[:])
        nc.sync.dma_start(bq_sb[:], bq[:])
        nc.sync.dma_start(bp_sb[:], bp[:])
        nc.sync.dma_start(vb_sb[:], vbias[:])
        nc.sync.dma_start(mask_sb[:], maskT[:])
        nc.sync.dma_start(wv_sb[:], Wv.rearrange("(o p) c -> p o c", p=P))

        # gpsimd queue: x context chunks (and later xq), first-use order
        x_tiles = {}
        for ci in range(2):
            t0 = ci * TC
            x_t = xin.tile([P, CB, TC], bf16, tag="x", name=f"x{ci}")
            nc.gpsimd.dma_start(
                x_t[:], xctxT[:, t0:t0 + TC].rearrange("(o p) t -> p o t", p=P))
            x_tiles[ci] = x_t

        # ones column of v (AV rides the softmax denominator in row 64)
        nc.vector.memset(v_sb[:, :, :, D:D + 1], 1.0)

        def copy_bias(out, psum, bias_col):
            nc.scalar.activation(out, psum,
                                 mybir.ActivationFunctionType.Identity,
                                 bias=bias_col)

        def k_group(ci, rb):
            """kT rows rb*128.. for token chunk ci (transposed layout)."""
            t0 = ci * TC
            ps = pmm.tile([P, TC], f32, tag="mm", name=f"k{ci}_{rb}")
            x_t = x_tiles[ci]
            for kc in range(CB):
                nc.tensor.matmul(
                    ps[:], wk_sb[:, kc, rb * P:(rb + 1) * P],
                    x_t[:, kc, :], start=(kc == 0), stop=(kc == CB - 1))
            copy_bias(kT_sb[:, rb, t0:t0 + TC], ps[:], bk_sb[:, rb:rb + 1])

        def v_group(ci, tb, cb2):
            """v natural layout [tok, (h, d)] for token block tb of chunk ci."""
            kb = (ci * TC) // P + tb
            ps = pmm.tile([P, TC], f32, tag="mm", name=f"v{ci}_{tb}_{cb2}")
            x_t = x_tiles[ci]
            for kc in range(CB):
                nc.tensor.matmul(
                    ps[:], x_t[:, kc, tb * P:(tb + 1) * P],
                    wv_sb[:, kc, cb2 * 512:(cb2 + 1) * 512],
                    start=(kc == 0), stop=(kc == CB - 1))
            h0 = cb2 * 8
            nc.vector.tensor_tensor(
                v_sb[:, h0:h0 + 8, kb, 0:D],
                ps.rearrange("p (h d) -> p h d", d=D),
                vb_sb[:, h0:h0 + 8, :], mybir.AluOpType.add)

        # ---------------- P0: K and V projections, tokens 0..1023 --------
        for ci in range(2):
            for rb in range(CB):
                k_group(ci, rb)
            for tb in range(TC // P):
                for cb2 in range(2):
                    v_group(ci, tb, cb2)

        # ---------------- P1: Q projection ----------------
        wq_tiles = {}

        def wq_dma(rb):
            wq_t = wpp.tile([P, CB, P], bf16, tag="w", name=f"wq{rb}")
            nc.sync.dma_start(
                wq_t[:], Wq[:, rb * P:(rb + 1) * P].rearrange(
                    "(o p) c -> p o c", p=P))
            wq_tiles[rb] = wq_t

        for qc in range(2):
            xq_t = xin.tile([P, CB, TC], bf16, tag="x", name=f"xq{qc}")
            nc.gpsimd.dma_start(
                xq_t[:],
                xqT[:, qc * TC:(qc + 1) * TC].rearrange(
                    "(o p) t -> p o t", p=P))
            x_tiles[f"q{qc}"] = xq_t
        for rb in range(2):
            wq_dma(rb)
        for rb in range(CB):
            if rb + 2 < CB:
                wq_dma(rb + 2)
            wq_t = wq_tiles.pop(rb)
            for qc in range(2):
                ps = pmm.tile([P, TC], f32, tag="mm", name=f"q{qc}_{rb}")
                for kc in range(CB):
                    nc.tensor.matmul(
                        ps[:], wq_t[:, kc, :], x_tiles[f"q{qc}"][:, kc, :],
                        start=(kc == 0), stop=(kc == CB - 1))
                copy_bias(qT_sb[:, rb, qc * TC:(qc + 1) * TC], ps[:],
                          bq_sb[:, rb:rb + 1])

        # x chunks for tokens 1024..2047 (S3 filler), reusing xin slots
        for ci in range(2, 4):
            t0 = ci * TC
            x_t = xin.tile([P, CB, TC], bf16, tag="x", name=f"x{ci}")
            nc.gpsimd.dma_start(
                x_t[:], xctxT[:, t0:t0 + TC].rearrange("(o p) t -> p o t", p=P))
            x_tiles[ci] = x_t

        # -------- S3/S4: attention steps + fillers (flat pipeline) ----
        L = [(0, h) for h in range(H)] + [(1, h) for h in range(H)]
        P_ts, py_ts, recips, bcs = {}, {}, {}, {}
        yT_tiles = {}

        def qstart(j, kb):
            return max(0, kb // 2 - 4 * j) * P

        # filler units: 32 proj groups for tokens 1024..2047 over steps 0..13
        proj_fill = []
        for ci in range(2, 4):
            for rb in range(CB):
                proj_fill.append(lambda ci=ci, rb=rb: k_group(ci, rb))
            for tb in range(TC // P):
                for cb2 in range(2):
                    proj_fill.append(
                        lambda ci=ci, tb=tb, cb2=cb2: v_group(ci, tb, cb2))

        wp_tiles = {}

        def wp_dma(j, ob):
            wp_t = wpp.tile([P, CB, P], bf16, tag="w", name=f"wp{j}_{ob}")
            nc.sync.dma_start(
                wp_t[:], Wp[:, ob * P:(ob + 1) * P].rearrange(
                    "(o p) c -> p o c", p=P))
            wp_tiles[(j, ob)] = wp_t

        def emit_outproj(j, ob):
            q0 = j * 512
            yT_sb = yT_tiles[j]
            wp_t = wp_tiles.pop((j, ob))
            po = pmm.tile([P, 512], f32, tag="mm", name=f"po{j}_{ob}")
            for yc in range(CB):
                nc.tensor.matmul(po[:], wp_t[:, yc, :], yT_sb[:, yc, :],
                                 start=(yc == 0), stop=(yc == CB - 1))
            o_sb = opool.tile([P, 512], f32, tag="o_sb")
            copy_bias(o_sb[:], po[:], bp_sb[:, ob:ob + 1])
            nc.sync.dma_start(outT[ob * P:(ob + 1) * P, q0:q0 + 512], o_sb[:])

        def sc_pair(cur, p_):
            """Scores for key-block pair p_ of (j, h) = cur, then exp (+mask)."""
            j, h = cur
            q0 = j * 512
            hp, hb = (h % 2) * D, h // 2
            P_t = P_ts[cur]
            qs = qstart(j, 2 * p_)
            ss = ps_s.tile([P, 1024], f32, tag="s")
            for dj in range(2):
                kb = 2 * p_ + dj
                nc.tensor.matmul(
                    ss[:, dj * 512 + qs:(dj + 1) * 512],
                    kT_sb[hp:hp + D, hb, kb * P:(kb + 1) * P],
                    qT_sb[hp:hp + D, hb, q0 + qs:q0 + 512],
                    start=True, stop=True)
            if qs == 0:
                nc.scalar.activation(
                    P_t.rearrange("p a b -> p (a b)")
                    [:, 2 * p_ * 512:(2 * p_ + 2) * 512],
                    ss[:], mybir.ActivationFunctionType.Exp, scale=SCALE)
            else:
                nc.scalar.activation(
                    P_t[:, 2 * p_:2 * p_ + 2, qs:512],
                    ss.rearrange("p (a b) -> p a b", a=2)[:, :, qs:512],
                    mybir.ActivationFunctionType.Exp, scale=SCALE)
            # causal diagonal mask for this pair (GpSimd, right after exp:
            # next step's AV never waits on a late mask batch)
            mq = p_ - 4 * j
            if mq >= 0:
                kb = 2 * p_
                sl = P_t[:, kb:kb + 2, mq * P:(mq + 1) * P]
                nc.gpsimd.tensor_mul(sl, sl, mask_sb[:])

        def av_share(prv, kb, last):
            jj, hh = prv
            avs = qstart(jj, kb)
            nc.tensor.matmul(
                py_ts[prv][:, avs:512], v_sb[:, hh, kb, :],
                P_ts[prv][:, kb, avs:512],
                start=(kb == 0), stop=last)

        for s in range(34):
            cur = L[s] if s < 32 else None
            prv = L[s - 1] if 1 <= s <= 32 else None
            pp2 = L[s - 2] if 2 <= s <= 33 else None

            # finalize head s-2: y = py[0:D] * (1/denom) broadcast, frees bank
            if pp2 is not None:
                jj, hh = pp2
                py = py_ts.pop(pp2)
                hp, hb = (hh % 2) * D, hh // 2
                nc.vector.tensor_mul(yT_tiles[jj][hp:hp + D, hb, :],
                                     py[0:D, :], bcs.pop(pp2)[:])

            # build the PE slot list for this step
            units = []
            if cur is not None:
                j, h = cur
                if h == 0:
                    yT_tiles[j] = ypool.tile([P, CB, 512], bf16, tag="yT",
                                             name=f"yT{j}")
                P_ts[cur] = ppool.tile([P, NKB, 512], bf16, tag="P",
                                       name=f"Pt{s}")
                units.extend(("sc", p_) for p_ in range(4 * j + 4))
            av_kbs = []
            if prv is not None:
                av_kbs = list(range(8 * prv[0] + 8))
                py_ts[prv] = ps_y.tile([D + 1, 512], f32, tag="y",
                                       name=f"py{s}")

            fillers = []
            if s < 16:
                fillers = proj_fill[(s * 32) // 16:((s + 1) * 32) // 16]
            elif 17 <= s <= 31:
                lo, hi = ((s - 17) * 8) // 15, ((s - 16) * 8) // 15
                for g in range(lo, hi):
                    if (0, g) not in wp_tiles:
                        wp_dma(0, g)
                    fillers.append(lambda g=g: emit_outproj(0, g))
                if hi < 8 and (0, hi) not in wp_tiles:
                    wp_dma(0, hi)

            # interleave: sc pairs spread across av shares, fillers between
            nsc = max(len(units), 1)
            slots = []
            for i, u in enumerate(units or [None]):
                if u is not None:
                    slots.append(u)
                lo = len(av_kbs) * i // nsc
                hi = len(av_kbs) * (i + 1) // nsc
                slots.extend(("av", kb) for kb in av_kbs[lo:hi])
            nf = len(fillers)
            if nf:
                stride = max(1, len(slots) // nf)
                out_slots = []
                fi = 0
                for i, u in enumerate(slots):
                    out_slots.append(u)
                    if i % stride == stride - 1 and fi < nf:
                        out_slots.append(("fill", fi))
                        fi += 1
                while fi < nf:
                    out_slots.append(("fill", fi))
                    fi += 1
                slots = out_slots

            for kind, arg in slots:
                if kind == "sc":
                    sc_pair(cur, arg)
                elif kind == "av":
                    av_share(prv, arg, last=(arg == len(av_kbs) - 1))
                else:
                    fillers[arg]()

            # normalization front-end for prv: 1/denom on DVE (18-bit approx),
            # broadcast across partitions on GpSimd. No PE/ScalarE involved.
            if prv is not None:
                # stage denominator row to partition 0 (custom-DVE recip
                # constants live at partition 0; PSUM reads must be aligned)
                den = nrm.tile([1, 512], f32, tag="den")
                nc.vector.tensor_copy(den[:], py_ts[prv][D:D + 1, :])
                recip = nrm.tile([1, 512], f32, tag="recip")
                nc.vector.reciprocal_approx_fast(recip[:], den[:])
                bc = bcp.tile([D, 512], f32, tag="bc")
                bcs[prv] = bc
                nc.gpsimd.partition_broadcast(bc[:], recip[:])

        # -------- S5: j1 output projection tail --------
        wp_dma(1, 0)
        wp_dma(1, 1)
        for ob in range(CB):
            if ob + 2 < CB:
                wp_dma(1, ob + 2)
            emit_outproj(1, ob)

    nc.compile()
    nc.m = get_hw_module(nc.m)
    return nc


def _prep_in_maps(x, mask, Wq, bq, Wk, bk, Wv, bv, Wp, bp):
    import ml_dtypes

    del mask  # causal structure is hardcoded (tril), verified upstream
    bf = ml_dtypes.bfloat16
    tobf = lambda a: np.ascontiguousarray(np.asarray(a, np.float32).astype(bf))
    Wq_h, Wk_h, Wv_h, Wp_h = tobf(Wq), tobf(Wk), tobf(Wv), tobf(Wp)
    b_col = lambda b: np.ascontiguousarray(
        np.asarray(b, np.float32).reshape(CB, P).T)
    bq_h, bk_h, bp_h = b_col(bq), b_col(bk), b_col(bp)
    vb_h = np.ascontiguousarray(np.broadcast_to(
        np.asarray(bv, np.float32).astype(bf).reshape(1, H, D), (P, H, D)))

    masks = []
    for par in range(2):
        c = np.arange(2 * P)[:, None]  # key offset within diagonal pair
        r_ = np.arange(P)[None, :]  # query offset within sub-block
        m = (c <= 2 * r_ + par).astype(np.float32).astype(bf)  # [256, 128]
        masks.append(np.ascontiguousarray(
            m.reshape(2, P, P).transpose(1, 0, 2)))

    in_maps = []
    for core in range(N_CORES):
        b, par = core // 2, core % 2
        xb = np.asarray(x[b], np.float32)
        in_maps.append({
            "xctxT": tobf(xb.T),
            "xqT": tobf(xb[par::2].T),
            "Wq": Wq_h, "Wk": Wk_h, "Wv": Wv_h, "Wp": Wp_h,
            "bq": bq_h, "bk": bk_h, "bp": bp_h,
            "vbias": vb_h, "maskT": masks[par],
        })
    return in_maps


def kernel(x, mask, Wq, bq, Wk, bk, Wv, bv, Wp, bp):
    from concourse import bass_utils

    if "nc" not in _CACHE:
        _CACHE["nc"] = _build_nc()
    nc = _CACHE["nc"]

    in_maps = _prep_in_maps(x, mask, Wq, bq, Wk, bk, Wv, bv, Wp, bp)
    res = bass_utils.run_bass_kernel_spmd(
        nc, in_maps, core_ids=list(range(N_CORES)))

    out = np.empty((B, T, C), np.float32)
    for core in range(N_CORES):
        b, par = core // 2, core % 2
        out[b, par::2, :] = res.results[core]["outT"].T
    return out
